# revision 27
# baseline (speedup 1.0000x reference)
"""Multi-head self-attention (B=2, S=2048, D=1024, H=16) on 8 trn2 cores.

Sharding: core c = b*4 + g  (b = batch, g = head-group of 4 heads).
Each core computes, for its batch b and heads 4g..4g+3:
  Qt/Kt = W^T x_b^T + bias   -> [128, 2048] feature-major per head pair
  V||ones                    -> vstk [128 keys, h, cp, 65]
  scoresT[k,q] per head      -> [128 keys, 1024 q] psum ([64,128] Kt weights)
  expT = exp(0.125*scoresT)  (ACT, straight from PSUM)
  ctxT/rowsum via PV matmul with [128, 65] V||ones weights (contract 128 keys)
  ctxT normalized by 1/rowsum (DMA partition-broadcast + DVE mult)
  y = ctx @ Wo               -> [2048, 1024]
One globally software-pipelined attention loop (128 iterations); pair-1
Q/K projections and the first-half output projection interleave into the
ACT-bound exp stream; tail output projection overlaps the last normalize
chain. Host: Y[b] = sum_g y_partial + (bo + bv @ Wo).
"""

import sys

sys.path.insert(0, "/opt/trn_rl_repo")

import numpy as np

import concourse.bass as bass
import concourse.bass_utils as _bass_utils
import concourse.mybir as mybir
import concourse.tile as tile

F32 = mybir.dt.float32
F16 = mybir.dt.float16
MMDT = F16                     # dtype for all matmul operands
AF = mybir.ActivationFunctionType

D = 1024          # d_model
S = 2048          # sequence length
HPC = 4           # heads per core
DK = 64           # head dim
E = HPC * DK      # 256 features per core
N_CORES = 8

KT = D // 128     # 8 k-tiles over d_model
CP = S // 128     # 16 key chunks of 128
ET = 2            # head pairs

# attention sections: (head, q-half), qh-outer
SECTIONS = [(h, qh) for qh in range(2) for h in range(HPC)]
NG = len(SECTIONS) * CP        # 128 global iterations


_ENGINE_OPS = {
    "InstMatmult", "InstActivation", "InstTensorCopy", "InstTensorTensor",
    "InstReciprocal", "InstTensorReduce", "InstMemset", "InstIota",
    "InstTensorScalarPtr", "InstTranspose", "InstLdweights",
    "InstDMACopy", "InstDrain", "InstNoOp",
}


def _dedup_ldweights(nc):
    """Consecutive matmuls often reuse identical weights (score j-halves, PV
    j-halves, proj q-chunks). The rust layer emits one standalone
    InstLdweights per matmul; reloading identical weights costs ~100ns of PE
    array drain each. Drop an InstLdweights when the previously loaded
    weights AP is byte-identical (weight tiles here are write-once), folding
    its waits into the next matmul (legalized afterwards)."""

    def key(ap):
        return (ap.memref, ap.offset, str(ap.ap), str(ap.dtype))

    n_drop = 0
    for f in nc.m.functions:
        for bb in f.blocks:
            out = []
            pending_waits = []
            last_w = None
            for i in bb.instructions:
                if type(i).__name__ == "InstLdweights":
                    k = key(i.ins[0])
                    if k == last_w:
                        si = getattr(i, "sync_info", None)
                        if si is not None and si.on_wait:
                            pending_waits.extend(si.on_wait)
                        n_drop += 1
                        continue
                    last_w = k
                elif pending_waits and type(i).__name__ == "InstMatmult":
                    si = getattr(i, "sync_info", None)
                    waits = list(si.on_wait) if si else []
                    upd = list(si.on_update) if si else []
                    i.sync_info = mybir.SyncInfo(
                        on_wait=pending_waits + waits, on_update=upd)
                    pending_waits = []
                out.append(i)
            assert not pending_waits
            bb.instructions = out
    return n_drop


def _legalize_matmul_waits(nc):
    """walrus allows at most 1 sync wait on engine compute instructions; Tile
    sometimes emits more. Move the excess onto EventSemaphore instructions
    (cap 2 each) placed immediately before in same-engine program order."""
    for f in nc.m.functions:
        for bb in f.blocks:
            out = []
            changed = False
            for i in bb.instructions:
                si = getattr(i, "sync_info", None)
                if (
                    type(i).__name__ in _ENGINE_OPS
                    and si is not None
                    and si.on_wait
                    and len(si.on_wait) > 1
                ):
                    waits = list(si.on_wait)
                    excess, keep = waits[:-1], waits[-1:]
                    for c in range(0, len(excess), 2):
                        ev = mybir.InstEventSemaphore(
                            name=f"{i.name}-mmw{c}", ins=[], outs=[]
                        )
                        ev.engine = i.engine
                        ev.sync_info = mybir.SyncInfo(
                            on_wait=excess[c:c + 2], on_update=[]
                        )
                        out.append(ev)
                    i.sync_info = mybir.SyncInfo(
                        on_wait=keep, on_update=list(si.on_update)
                    )
                    changed = True
                out.append(i)
            if changed:
                bb.instructions = out


def build_nc():
    nc = bass.Bass()

    xt = nc.dram_tensor("xt", [D, S], MMDT, kind="ExternalInput")
    wq = nc.dram_tensor("wq", [D, E], MMDT, kind="ExternalInput")
    wk = nc.dram_tensor("wk", [D, E], MMDT, kind="ExternalInput")
    wv = nc.dram_tensor("wv", [D, E], MMDT, kind="ExternalInput")
    wo = nc.dram_tensor("wo", [E, D], MMDT, kind="ExternalInput")
    bq = nc.dram_tensor("bq", [E], F32, kind="ExternalInput")
    bk = nc.dram_tensor("bk", [E], F32, kind="ExternalInput")
    y = nc.dram_tensor("y", [S, D], F32, kind="ExternalOutput")

    with tile.TileContext(nc) as tc:
        with tc.tile_pool(name="persist", bufs=1) as pp:
            # ---- persistent tiles ----
            qt_sb = [pp.tile([128, S], MMDT, tag=f"qt{t}", name=f"qt{t}")
                     for t in range(ET)]
            ktp = [pp.tile([128, S], MMDT, tag=f"ktp{t}", name=f"ktp{t}")
                   for t in range(ET)]
            # vstk[:, h, cp, 0:64] = V rows for head h, keys cp*128..+128
            # (partition = key within chunk); [:, h, cp, 64] = 1.0 (rowsum).
            vstk = pp.tile([128, HPC, CP, DK + 1], MMDT, tag="vstk")
            ctx_sb = [pp.tile([128, S], MMDT, tag=f"ctx{t}", name=f"ctx{t}")
                      for t in range(ET)]
            wo_sb = [pp.tile([128, D], MMDT, tag=f"wo{t}", name=f"wo{t}")
                     for t in range(ET)]
            bq_sb = pp.tile([128, ET], F32, tag="bq")
            bk_sb = pp.tile([128, ET], F32, tag="bk")
            xt_sb = [pp.tile([128, S], MMDT, tag=f"xt{k}", name=f"xt{k}")
                     for k in range(KT)]
            wq_sb = pp.tile([128, KT, E], MMDT, tag="wq")
            wk_sb = pp.tile([128, KT, E], MMDT, tag="wk")
            wv_sb = pp.tile([128, KT, E], MMDT, tag="wv")

            nc.gpsimd.dma_start(bq_sb, bq.rearrange("(t p) -> p t", p=128))
            nc.gpsimd.dma_start(bk_sb, bk.rearrange("(t p) -> p t", p=128))
            # k-layer inputs arrive together so QK0 proj chases the DMAs;
            # issues spread across four queues for parallel transfer.
            for k in range(KT):
                nc.scalar.dma_start(wq_sb[:, k, :],
                                    wq[k * 128:(k + 1) * 128, :])
                nc.scalar.dma_start(wk_sb[:, k, :],
                                    wk[k * 128:(k + 1) * 128, :])
                nc.sync.dma_start(xt_sb[k][:, 0:1024],
                                  xt[k * 128:(k + 1) * 128, 0:1024])
                nc.gpsimd.dma_start(xt_sb[k][:, 1024:2048],
                                    xt[k * 128:(k + 1) * 128, 1024:2048])
            for k in range(KT):
                nc.scalar.dma_start(wv_sb[:, k, :],
                                    wv[k * 128:(k + 1) * 128, :])
            for t in range(ET):
                nc.scalar.dma_start(wo_sb[t], wo[t * 128:(t + 1) * 128, :])

            ones_sb = pp.tile([128, CP], F32, tag="ones")
            nc.vector.memset(ones_sb, 1.0)
            for h in range(HPC):
                nc.vector.tensor_copy(vstk[:, h, :, DK:DK + 1],
                                      ones_sb[:, :, None])

            # ---- stage A: pair-0 Q/K projections + V (all heads) ----
            with tc.tile_pool(name="psA", bufs=4, space="PSUM") as psA:
                # 4 resident [128,1024] psums (8 banks), k-outer so matmuls
                # chase the input DMAs.
                qps = [psA.tile([128, 1024], F32, tag="proj", name=f"qp{i}")
                       for i in range(2)]
                kps = [psA.tile([128, 1024], F32, tag="proj", name=f"kp{i}")
                       for i in range(2)]
                for k in range(KT):
                    for half in range(2):
                        for j in range(2):
                            q0 = half * 1024 + j * 512
                            nc.tensor.matmul(
                                qps[half][:, j * 512:(j + 1) * 512],
                                wq_sb[:, k, 0:128],
                                xt_sb[k][:, q0:q0 + 512],
                                start=(k == 0), stop=(k == KT - 1),
                            )
                    for half in range(2):
                        for j in range(2):
                            q0 = half * 1024 + j * 512
                            nc.tensor.matmul(
                                kps[half][:, j * 512:(j + 1) * 512],
                                wk_sb[:, k, 0:128],
                                xt_sb[k][:, q0:q0 + 512],
                                start=(k == 0), stop=(k == KT - 1),
                            )
                # evictions with bias on ACT (idle during stage A)
                for half in range(2):
                    nc.scalar.activation(
                        qt_sb[0][:, half * 1024:(half + 1) * 1024],
                        qps[half], AF.Identity, bias=bq_sb[:, 0:1])
                for half in range(2):
                    nc.scalar.activation(
                        ktp[0][:, half * 1024:(half + 1) * 1024],
                        kps[half], AF.Identity, bias=bk_sb[:, 0:1])

                # V for all 4 heads, seq-major: psum [128 seq, 256 feat]
                for s in range(CP):
                    vp = psA.tile([128, 1024], F32, tag="proj",
                                  name=f"vp{s}")
                    for k in range(KT):
                        nc.tensor.matmul(
                            vp[:, 0:E],
                            xt_sb[k][:, s * 128:(s + 1) * 128],
                            wv_sb[:, k, :],
                            start=(k == 0), stop=(k == KT - 1),
                        )
                    nc.vector.tensor_copy(
                        vstk[:, :, s, 0:DK],
                        vp[:, 0:E].rearrange("p (h d) -> p h d", d=DK))

            # ---- stage B: one pipelined attention loop ----
            with (
                tc.tile_pool(name="stageB", bufs=3) as pb,
                tc.tile_pool(name="dramB", bufs=3, space="DRAM") as dramB,
            ):
              with (
                tc.tile_pool(name="psS", bufs=2, space="PSUM") as psS,
                tc.tile_pool(name="psC", bufs=2, space="PSUM") as psC,
                tc.tile_pool(name="psX", bufs=2, space="PSUM") as psX,
              ):
                # --- feeder jobs: closures emitting one PE matmul each ---
                def proj1_jobs(w_sb, b_col, dst, chunks):
                    """Pair-1 proj matmuls, k-inner per [128,512] chunk."""
                    jobs = []
                    for ch in chunks:
                        ps = {}

                        def mk(k, ch=ch, ps=ps):
                            def go():
                                if k == 0:
                                    ps[0] = psX.tile(
                                        [128, 512], F32, tag="aux",
                                        name=f"pj{id(w_sb)}_{ch}")
                                nc.tensor.matmul(
                                    ps[0],
                                    w_sb[:, k, 128:256],
                                    xt_sb[k][:, ch * 512:(ch + 1) * 512],
                                    start=(k == 0), stop=(k == KT - 1),
                                )
                                if k == KT - 1:
                                    nc.vector.tensor_scalar_add(
                                        dst[:, ch * 512:(ch + 1) * 512],
                                        ps[0], b_col)
                            return go
                        for k in range(KT):
                            jobs.append(mk(k))
                    return jobs

                def outproj_jobs(tiles):
                    """Output projection per 128-q tile: 4 matmuls + evict
                    + y DMA."""
                    jobs = []
                    for qt_i in tiles:
                        st = {}

                        def mk(t, n, qt_i=qt_i, st=st):
                            def go():
                                if (t, n) == (0, 0):
                                    st['ps'] = [
                                        psX.tile([128, 512], F32, tag="aux",
                                                 name=f"yp{qt_i}_{j}")
                                        for j in range(2)]
                                    st['ys'] = pb.tile(
                                        [128, 1024], F32, tag="ys",
                                        name=f"ys{qt_i}", bufs=6)
                                nc.tensor.matmul(
                                    st['ps'][n],
                                    ctx_sb[t][:, qt_i * 128:(qt_i + 1) * 128],
                                    wo_sb[t][:, n * 512:(n + 1) * 512],
                                    start=(t == 0), stop=(t == ET - 1),
                                )
                                if t == ET - 1:
                                    nc.vector.tensor_copy(
                                        st['ys'][:, n * 512:(n + 1) * 512],
                                        st['ps'][n])
                                if (t, n) == (1, 1):
                                    nc.sync.dma_start(
                                        y[qt_i * 128:(qt_i + 1) * 128, :],
                                        st['ys'])
                            return go
                        for t in range(ET):
                            for n in range(2):
                                jobs.append(mk(t, n))
                    return jobs

                # schedule feeder jobs onto global iterations
                schedule = [[] for _ in range(NG)]

                def assign(g_lo, g_hi, jobs):
                    n_slots = g_hi - g_lo
                    acc = 0.0
                    i = 0
                    per = len(jobs) / n_slots
                    for g in range(g_lo, g_hi):
                        acc += per
                        take = int(round(acc)) - i
                        schedule[g] = jobs[i:i + take]
                        i += take
                    assert i == len(jobs)

                # Q1 half0 + K1 during sections (0,0),(1,0); order matters:
                # section (2,0)'s first scores (emitted at g=31) need the
                # Q1-h0 and K1-ch0 evictions done well before.
                assign(0, 32,
                       proj1_jobs(wq_sb, bq_sb[:, 1:2], qt_sb[1], range(2))
                       + proj1_jobs(wk_sb, bk_sb[:, 1:2], ktp[1], range(4)))
                # Q1 half1 during (2,0),(3,0)
                assign(32, 64,
                       proj1_jobs(wq_sb, bq_sb[:, 1:2], qt_sb[1],
                                  range(2, 4)))
                # outproj q-half0 during (0,1),(1,1); offset past (3,0)'s
                # normalize chain (its ctx lands a few us into (0,1)).
                assign(72, 96, outproj_jobs(range(8)))

                def emit_scores(g):
                    sec, cp = divmod(g, CP)
                    h, qh = SECTIONS[sec]
                    t, hp = h // 2, h % 2
                    sc_ps = psS.tile([128, 1024], F32, tag="sc",
                                     name=f"sc{g}")
                    for j in range(2):
                        nc.tensor.matmul(
                            sc_ps[:, j * 512:(j + 1) * 512],
                            ktp[t][hp * 64:hp * 64 + 64,
                                   cp * 128:(cp + 1) * 128],
                            qt_sb[t][hp * 64:hp * 64 + 64,
                                     qh * 1024 + j * 512:
                                     qh * 1024 + (j + 1) * 512],
                            start=True, stop=True,
                        )
                    return sc_ps

                def section_end(h, qh, ctx_ps):
                    t, hp = h // 2, h % 2
                    stg = pb.tile([DK + 1, 1024], F32, tag="stg",
                                  name=f"stg{h}_{qh}", bufs=3)
                    for j in range(2):
                        nc.vector.tensor_copy(
                            stg[:, j * 512:(j + 1) * 512], ctx_ps[j])
                    # reciprocal of rowsum via DRAM scatter to 64 partitions
                    # ([1,1024] single-lane DVE reciprocal is ~6.5us; this
                    # chain is ~3us and hidden under the next section).
                    rc_dr = dramB.tile([1, 1024], F32, tag="rc_dr",
                                       name=f"rcdr{h}_{qh}")
                    # single SBUF->SBUF partition-scatter DMA (dst/src APs
                    # iterate element-wise) replaces the two-hop DRAM bounce
                    rs64 = pb.tile([64, 16], F32, tag="rs64",
                                   name=f"rs64{h}_{qh}")
                    nc.sync.dma_start(rs64, stg[DK:DK + 1, :])
                    rc64 = pb.tile([64, 16], F32, tag="rc64",
                                   name=f"rc64{h}_{qh}")
                    nc.vector.reciprocal(rc64, rs64)
                    nc.sync.dma_start(
                        rc_dr.rearrange("o (p f) -> (o p) f", f=16), rc64)
                    rb = pb.tile([64, 1024], F32, tag="rb",
                                 name=f"rb{h}_{qh}")
                    nc.sync.dma_start(rb, rc_dr.to_broadcast([64, 1024]))
                    nc.vector.tensor_mul(
                        ctx_sb[t][hp * 64:hp * 64 + 64,
                                  qh * 1024:(qh + 1) * 1024],
                        stg[0:DK, :],
                        rb,
                    )

                ctx_ps = None
                sc_cur = emit_scores(0)
                for g in range(NG):
                    sec, cp = divmod(g, CP)
                    h, qh = SECTIONS[sec]
                    if cp == 0:
                        ctx_ps = [psC.tile([DK + 1, 512], F32, tag="ctx",
                                           name=f"cx{sec}_{j}")
                                  for j in range(2)]
                    sc_next = emit_scores(g + 1) if g + 1 < NG else None
                    ex = pb.tile([128, 1024], MMDT, tag="ex",
                                 name=f"ex{g}", bufs=4)
                    nc.scalar.activation(ex, sc_cur, AF.Exp, scale=0.125)
                    for job in schedule[g]:
                        job()
                    for j in range(2):
                        nc.tensor.matmul(
                            ctx_ps[j],
                            vstk[:, h, cp, :],
                            ex[:, j * 512:(j + 1) * 512],
                            start=(cp == 0), stop=(cp == CP - 1),
                        )
                    if cp == CP - 1:
                        section_end(h, qh, ctx_ps)
                    sc_cur = sc_next

              # ---- tail: output projection for q-half 1 ----
              # Separate psum pool (attention pools closed -> banks free).
              # All eight tiles' t=0 matmuls run during the final normalize
              # chain (PE busy/warm); tiles 8-11 stay resident in psum,
              # tiles 12-15 stage to SBUF via the now-idle ACT engine and
              # re-accumulate with a DVE add after t=1.
              with tc.tile_pool(name="psT", bufs=4, space="PSUM") as psT:
                def t_mm(yp, t, qt_i, start, stop):
                    for n in range(2):
                        nc.tensor.matmul(
                            yp[:, n * 512:(n + 1) * 512],
                            ctx_sb[t][:, qt_i * 128:(qt_i + 1) * 128],
                            wo_sb[t][:, n * 512:(n + 1) * 512],
                            start=start, stop=stop,
                        )

                yps = {}
                y0_sb = {}
                for qt_i in range(8, 10):
                    yp = psT.tile([128, 1024], F32, tag="yt",
                                  name=f"yt{qt_i}", bufs=2)
                    yps[qt_i] = yp
                    t_mm(yp, 0, qt_i, start=True, stop=False)
                for qt_i in range(10, 16):
                    yp = psT.tile([128, 1024], F32, tag="yt0",
                                  name=f"yt0_{qt_i}", bufs=2)
                    t_mm(yp, 0, qt_i, start=True, stop=True)
                    ys0 = pb.tile([128, 1024], F32, tag="ys0",
                                  name=f"ys0_{qt_i}", bufs=6)
                    y0_sb[qt_i] = ys0
                    if qt_i % 2 == 0:
                        nc.scalar.copy(ys0, yp)
                    else:
                        nc.vector.tensor_copy(ys0, yp)

                for i, qt_i in enumerate(range(8, 10)):
                    yp = yps[qt_i]
                    t_mm(yp, 1, qt_i, start=False, stop=True)
                    ys = pb.tile([128, 1024], F32, tag="ys",
                                 name=f"ys{qt_i}", bufs=6)
                    if i % 2 == 0:
                        nc.scalar.copy(ys, yp)
                    else:
                        nc.vector.tensor_copy(ys, yp)
                    rows = slice(qt_i * 128, (qt_i + 1) * 128)
                    nc.sync.dma_start(y[rows, 0:512], ys[:, 0:512])
                    nc.gpsimd.dma_start(y[rows, 512:1024], ys[:, 512:1024])
                for i, qt_i in enumerate(range(10, 16)):
                    yp = psT.tile([128, 1024], F32, tag="yt0",
                                  name=f"yt1_{qt_i}", bufs=2)
                    t_mm(yp, 1, qt_i, start=True, stop=True)
                    ys = pb.tile([128, 1024], F32, tag="ys",
                                 name=f"ys{qt_i}", bufs=6)
                    nc.vector.tensor_add(ys, yp, y0_sb[qt_i])
                    rows = slice(qt_i * 128, (qt_i + 1) * 128)
                    nc.sync.dma_start(y[rows, 0:512], ys[:, 0:512])
                    nc.gpsimd.dma_start(y[rows, 512:1024], ys[:, 512:1024])
    _dedup_ldweights(nc)
    _legalize_matmul_waits(nc)
    return nc


_NC_CACHE = None


def _get_nc():
    global _NC_CACHE
    if _NC_CACHE is None:
        _NC_CACHE = build_nc()
    return _NC_CACHE


def make_in_maps(inputs):
    mmnp = mybir.dt.np(MMDT)
    x = np.asarray(inputs["x"], dtype=np.float32)
    Wq = np.asarray(inputs["Wq"], dtype=np.float32)
    Wk = np.asarray(inputs["Wk"], dtype=np.float32)
    Wv = np.asarray(inputs["Wv"], dtype=np.float32)
    Wo = np.asarray(inputs["Wo"], dtype=np.float32)
    bq = np.asarray(inputs["bq"], dtype=np.float32)
    bk = np.asarray(inputs["bk"], dtype=np.float32)

    in_maps = []
    for c in range(N_CORES):
        b, g = c // 4, c % 4
        sl = slice(g * E, (g + 1) * E)
        in_maps.append({
            "xt": np.ascontiguousarray(x[b].T).astype(mmnp),
            "wq": np.ascontiguousarray(Wq[:, sl]).astype(mmnp),
            "wk": np.ascontiguousarray(Wk[:, sl]).astype(mmnp),
            "wv": np.ascontiguousarray(Wv[:, sl]).astype(mmnp),
            "wo": np.ascontiguousarray(Wo[sl, :]).astype(mmnp),
            "bq": np.ascontiguousarray(bq[sl]),
            "bk": np.ascontiguousarray(bk[sl]),
        })
    return in_maps


def kernel(x, Wq, bq, Wk, bk, Wv, bv, Wo, bo):
    from concourse.bass_utils import run_bass_kernel_spmd

    x = np.asarray(x, dtype=np.float32)
    Wv = np.asarray(Wv, dtype=np.float32)
    Wo = np.asarray(Wo, dtype=np.float32)
    bv = np.asarray(bv, dtype=np.float32)
    bo = np.asarray(bo, dtype=np.float32)

    B = x.shape[0]
    nc = _get_nc()
    in_maps = make_in_maps({
        "x": x, "Wq": Wq, "Wk": Wk, "Wv": Wv, "Wo": Wo, "bq": bq, "bk": bk,
    })

    res = run_bass_kernel_spmd(nc, in_maps, core_ids=list(range(N_CORES)))

    bias_total = bo + bv @ Wo  # [D]
    out = np.zeros((B, S, D), dtype=np.float32)
    for c in range(N_CORES):
        out[c // 4] += res.results[c]["y"]
    out += bias_total[None, None, :]
    return out


# revision 31
# speedup vs baseline: 1.0006x; 1.0006x over previous
"""Multi-head self-attention (B=2, S=2048, D=1024, H=16) on 8 trn2 cores.

Sharding: core c = b*4 + g  (b = batch, g = head-group of 4 heads).
Each core computes, for its batch b and heads 4g..4g+3:
  Qt/Kt = W^T x_b^T + bias   -> [128, 2048] feature-major per head pair
  V||ones                    -> vstk [128 keys, h, cp, 65]
  scoresT[k,q] per head      -> [128 keys, 1024 q] psum ([64,128] Kt weights)
  expT = exp(0.125*scoresT)  (ACT, straight from PSUM)
  ctxT/rowsum via PV matmul with [128, 65] V||ones weights (contract 128 keys)
  ctxT normalized by 1/rowsum (DMA partition-broadcast + DVE mult)
  y = ctx @ Wo               -> [2048, 1024]
One globally software-pipelined attention loop (128 iterations); pair-1
Q/K projections and the first-half output projection interleave into the
ACT-bound exp stream; tail output projection overlaps the last normalize
chain. Host: Y[b] = sum_g y_partial + (bo + bv @ Wo).
"""

import sys

sys.path.insert(0, "/opt/trn_rl_repo")

import numpy as np

import concourse.bass as bass
import concourse.bass_utils as _bass_utils
import concourse.mybir as mybir
import concourse.tile as tile

F32 = mybir.dt.float32
F16 = mybir.dt.float16
MMDT = F16                     # dtype for all matmul operands
AF = mybir.ActivationFunctionType

D = 1024          # d_model
S = 2048          # sequence length
HPC = 4           # heads per core
DK = 64           # head dim
E = HPC * DK      # 256 features per core
N_CORES = 8

KT = D // 128     # 8 k-tiles over d_model
CP = S // 128     # 16 key chunks of 128
ET = 2            # head pairs

# attention sections: (head, q-half), qh-outer
SECTIONS = [(h, qh) for qh in range(2) for h in range(HPC)]
NG = len(SECTIONS) * CP        # 128 global iterations


_ENGINE_OPS = {
    "InstMatmult", "InstActivation", "InstTensorCopy", "InstTensorTensor",
    "InstReciprocal", "InstTensorReduce", "InstMemset", "InstIota",
    "InstTensorScalarPtr", "InstTranspose", "InstLdweights",
    "InstDMACopy", "InstDrain", "InstNoOp",
}


def _dedup_ldweights(nc):
    """Consecutive matmuls often reuse identical weights (score j-halves, PV
    j-halves, proj q-chunks). The rust layer emits one standalone
    InstLdweights per matmul; reloading identical weights costs ~100ns of PE
    array drain each. Drop an InstLdweights when the previously loaded
    weights AP is byte-identical (weight tiles here are write-once), folding
    its waits into the next matmul (legalized afterwards)."""

    def key(ap):
        return (ap.memref, ap.offset, str(ap.ap), str(ap.dtype))

    n_drop = 0
    for f in nc.m.functions:
        for bb in f.blocks:
            out = []
            pending_waits = []
            last_w = None
            for i in bb.instructions:
                if type(i).__name__ == "InstLdweights":
                    k = key(i.ins[0])
                    if k == last_w:
                        si = getattr(i, "sync_info", None)
                        if si is not None and si.on_wait:
                            pending_waits.extend(si.on_wait)
                        n_drop += 1
                        continue
                    last_w = k
                elif pending_waits and type(i).__name__ == "InstMatmult":
                    si = getattr(i, "sync_info", None)
                    waits = list(si.on_wait) if si else []
                    upd = list(si.on_update) if si else []
                    i.sync_info = mybir.SyncInfo(
                        on_wait=pending_waits + waits, on_update=upd)
                    pending_waits = []
                out.append(i)
            assert not pending_waits
            bb.instructions = out
    return n_drop


def _legalize_matmul_waits(nc):
    """walrus allows at most 1 sync wait on engine compute instructions; Tile
    sometimes emits more. Move the excess onto EventSemaphore instructions
    (cap 2 each) placed immediately before in same-engine program order."""
    for f in nc.m.functions:
        for bb in f.blocks:
            out = []
            changed = False
            for i in bb.instructions:
                si = getattr(i, "sync_info", None)
                if (
                    type(i).__name__ in _ENGINE_OPS
                    and si is not None
                    and si.on_wait
                    and len(si.on_wait) > 1
                ):
                    waits = list(si.on_wait)
                    excess, keep = waits[:-1], waits[-1:]
                    for c in range(0, len(excess), 2):
                        ev = mybir.InstEventSemaphore(
                            name=f"{i.name}-mmw{c}", ins=[], outs=[]
                        )
                        ev.engine = i.engine
                        ev.sync_info = mybir.SyncInfo(
                            on_wait=excess[c:c + 2], on_update=[]
                        )
                        out.append(ev)
                    i.sync_info = mybir.SyncInfo(
                        on_wait=keep, on_update=list(si.on_update)
                    )
                    changed = True
                out.append(i)
            if changed:
                bb.instructions = out


def build_nc():
    nc = bass.Bass()

    xt = nc.dram_tensor("xt", [D, S], MMDT, kind="ExternalInput")
    wq = nc.dram_tensor("wq", [D, E], MMDT, kind="ExternalInput")
    wk = nc.dram_tensor("wk", [D, E], MMDT, kind="ExternalInput")
    wv = nc.dram_tensor("wv", [D, E], MMDT, kind="ExternalInput")
    wo = nc.dram_tensor("wo", [E, D], MMDT, kind="ExternalInput")
    bq = nc.dram_tensor("bq", [E], F32, kind="ExternalInput")
    bk = nc.dram_tensor("bk", [E], F32, kind="ExternalInput")
    y = nc.dram_tensor("y", [S, D], F32, kind="ExternalOutput")

    with tile.TileContext(nc) as tc:
        with tc.tile_pool(name="persist", bufs=1) as pp:
            # ---- persistent tiles ----
            qt_sb = [pp.tile([128, S], MMDT, tag=f"qt{t}", name=f"qt{t}")
                     for t in range(ET)]
            ktp = [pp.tile([128, S], MMDT, tag=f"ktp{t}", name=f"ktp{t}")
                   for t in range(ET)]
            # vstk[:, h, cp, 0:64] = V rows for head h, keys cp*128..+128
            # (partition = key within chunk); [:, h, cp, 64] = 1.0 (rowsum).
            vstk = pp.tile([128, HPC, CP, DK + 1], MMDT, tag="vstk")
            ctx_sb = [pp.tile([128, S], MMDT, tag=f"ctx{t}", name=f"ctx{t}")
                      for t in range(ET)]
            wo_sb = [pp.tile([128, D], MMDT, tag=f"wo{t}", name=f"wo{t}")
                     for t in range(ET)]
            bq_sb = pp.tile([128, ET], F32, tag="bq")
            bk_sb = pp.tile([128, ET], F32, tag="bk")
            xt_sb = [pp.tile([128, S], MMDT, tag=f"xt{k}", name=f"xt{k}")
                     for k in range(KT)]
            wq_sb = pp.tile([128, KT, E], MMDT, tag="wq")
            wk_sb = pp.tile([128, KT, E], MMDT, tag="wk")
            wv_sb = pp.tile([128, KT, E], MMDT, tag="wv")

            nc.gpsimd.dma_start(bq_sb, bq.rearrange("(t p) -> p t", p=128))
            nc.gpsimd.dma_start(bk_sb, bk.rearrange("(t p) -> p t", p=128))
            # k-layer inputs arrive together so QK0 proj chases the DMAs;
            # issues spread across four queues for parallel transfer.
            for k in range(KT):
                nc.scalar.dma_start(wq_sb[:, k, :],
                                    wq[k * 128:(k + 1) * 128, :])
                nc.scalar.dma_start(wk_sb[:, k, :],
                                    wk[k * 128:(k + 1) * 128, :])
                nc.sync.dma_start(xt_sb[k][:, 0:1024],
                                  xt[k * 128:(k + 1) * 128, 0:1024])
                nc.gpsimd.dma_start(xt_sb[k][:, 1024:2048],
                                    xt[k * 128:(k + 1) * 128, 1024:2048])
            for k in range(KT):
                nc.scalar.dma_start(wv_sb[:, k, :],
                                    wv[k * 128:(k + 1) * 128, :])
            for t in range(ET):
                nc.scalar.dma_start(wo_sb[t], wo[t * 128:(t + 1) * 128, :])

            ones_sb = pp.tile([128, CP], F32, tag="ones")
            nc.vector.memset(ones_sb, 1.0)
            for h in range(HPC):
                nc.vector.tensor_copy(vstk[:, h, :, DK:DK + 1],
                                      ones_sb[:, :, None])
            # warm the ACT function table during the DMA dead time; the lazy
            # load (1.3us) otherwise lands on phase A's eviction critical path
            tbl_warm = pp.tile([1, 16], F32, tag="tblw")
            nc.scalar.activation(tbl_warm, ones_sb[0:1, 0:16], AF.Identity)

            # ---- stage A: pair-0 Q/K projections + V (all heads) ----
            with tc.tile_pool(name="psA", bufs=4, space="PSUM") as psA:
                # 4 resident [128,1024] psums (8 banks), k-outer so matmuls
                # chase the input DMAs.
                qps = [psA.tile([128, 1024], F32, tag="proj", name=f"qp{i}")
                       for i in range(2)]
                kps = [psA.tile([128, 1024], F32, tag="proj", name=f"kp{i}")
                       for i in range(2)]
                for k in range(KT):
                    for half in range(2):
                        for j in range(2):
                            q0 = half * 1024 + j * 512
                            nc.tensor.matmul(
                                qps[half][:, j * 512:(j + 1) * 512],
                                wq_sb[:, k, 0:128],
                                xt_sb[k][:, q0:q0 + 512],
                                start=(k == 0), stop=(k == KT - 1),
                            )
                    for half in range(2):
                        for j in range(2):
                            q0 = half * 1024 + j * 512
                            nc.tensor.matmul(
                                kps[half][:, j * 512:(j + 1) * 512],
                                wk_sb[:, k, 0:128],
                                xt_sb[k][:, q0:q0 + 512],
                                start=(k == 0), stop=(k == KT - 1),
                            )
                # evictions with bias on ACT (idle during stage A)
                for half in range(2):
                    nc.scalar.activation(
                        qt_sb[0][:, half * 1024:(half + 1) * 1024],
                        qps[half], AF.Identity, bias=bq_sb[:, 0:1])
                for half in range(2):
                    nc.scalar.activation(
                        ktp[0][:, half * 1024:(half + 1) * 1024],
                        kps[half], AF.Identity, bias=bk_sb[:, 0:1])

                # V for all 4 heads, seq-major: psum [128 seq, 256 feat]
                for s in range(CP):
                    vp = psA.tile([128, 1024], F32, tag="proj",
                                  name=f"vp{s}")
                    for k in range(KT):
                        nc.tensor.matmul(
                            vp[:, 0:E],
                            xt_sb[k][:, s * 128:(s + 1) * 128],
                            wv_sb[:, k, :],
                            start=(k == 0), stop=(k == KT - 1),
                        )
                    nc.vector.tensor_copy(
                        vstk[:, :, s, 0:DK],
                        vp[:, 0:E].rearrange("p (h d) -> p h d", d=DK))

            # ---- stage B: one pipelined attention loop ----
            with (
                tc.tile_pool(name="stageB", bufs=3) as pb,
                tc.tile_pool(name="dramB", bufs=3, space="DRAM") as dramB,
            ):
              with (
                tc.tile_pool(name="psS", bufs=2, space="PSUM") as psS,
                tc.tile_pool(name="psC", bufs=2, space="PSUM") as psC,
                tc.tile_pool(name="psX", bufs=2, space="PSUM") as psX,
              ):
                # --- feeder jobs: closures emitting one PE matmul each ---
                def proj1_jobs(w_sb, b_col, dst, chunks):
                    """Pair-1 proj matmuls, k-inner per [128,512] chunk."""
                    jobs = []
                    for ch in chunks:
                        ps = {}

                        def mk(k, ch=ch, ps=ps):
                            def go():
                                if k == 0:
                                    ps[0] = psX.tile(
                                        [128, 512], F32, tag="aux",
                                        name=f"pj{id(w_sb)}_{ch}")
                                nc.tensor.matmul(
                                    ps[0],
                                    w_sb[:, k, 128:256],
                                    xt_sb[k][:, ch * 512:(ch + 1) * 512],
                                    start=(k == 0), stop=(k == KT - 1),
                                )
                                if k == KT - 1:
                                    nc.vector.tensor_scalar_add(
                                        dst[:, ch * 512:(ch + 1) * 512],
                                        ps[0], b_col)
                            return go
                        for k in range(KT):
                            jobs.append(mk(k))
                    return jobs

                def outproj_jobs(tiles):
                    """Output projection per 128-q tile: 4 matmuls + evict
                    + y DMA."""
                    jobs = []
                    for qt_i in tiles:
                        st = {}

                        def mk(t, n, qt_i=qt_i, st=st):
                            def go():
                                if (t, n) == (0, 0):
                                    st['ps'] = [
                                        psX.tile([128, 512], F32, tag="aux",
                                                 name=f"yp{qt_i}_{j}")
                                        for j in range(2)]
                                    st['ys'] = pb.tile(
                                        [128, 1024], F32, tag="ys",
                                        name=f"ys{qt_i}", bufs=6)
                                nc.tensor.matmul(
                                    st['ps'][n],
                                    ctx_sb[t][:, qt_i * 128:(qt_i + 1) * 128],
                                    wo_sb[t][:, n * 512:(n + 1) * 512],
                                    start=(t == 0), stop=(t == ET - 1),
                                )
                                if t == ET - 1:
                                    nc.vector.tensor_copy(
                                        st['ys'][:, n * 512:(n + 1) * 512],
                                        st['ps'][n])
                                if (t, n) == (1, 1):
                                    nc.sync.dma_start(
                                        y[qt_i * 128:(qt_i + 1) * 128, :],
                                        st['ys'])
                            return go
                        for t in range(ET):
                            for n in range(2):
                                jobs.append(mk(t, n))
                    return jobs

                # schedule feeder jobs onto global iterations
                schedule = [[] for _ in range(NG)]

                def assign(g_lo, g_hi, jobs):
                    n_slots = g_hi - g_lo
                    acc = 0.0
                    i = 0
                    per = len(jobs) / n_slots
                    for g in range(g_lo, g_hi):
                        acc += per
                        take = int(round(acc)) - i
                        schedule[g] = jobs[i:i + take]
                        i += take
                    assert i == len(jobs)

                # Q1 half0 + K1 during sections (0,0),(1,0); order matters:
                # section (2,0)'s first scores (emitted at g=31) need the
                # Q1-h0 and K1-ch0 evictions done well before.
                assign(0, 32,
                       proj1_jobs(wq_sb, bq_sb[:, 1:2], qt_sb[1], range(2))
                       + proj1_jobs(wk_sb, bk_sb[:, 1:2], ktp[1], range(4)))
                # Q1 half1 during (2,0),(3,0)
                assign(32, 64,
                       proj1_jobs(wq_sb, bq_sb[:, 1:2], qt_sb[1],
                                  range(2, 4)))
                # outproj q-half0 tiles 0-4 during (0,1),(1,1); offset past
                # (3,0)'s normalize chain (its ctx lands a few us into
                # (0,1)). Tiles 5-7 fill the final normalize-chain gap in
                # the tail instead.
                assign(72, 96, outproj_jobs(range(5)))

                def emit_scores(g):
                    sec, cp = divmod(g, CP)
                    h, qh = SECTIONS[sec]
                    t, hp = h // 2, h % 2
                    sc_ps = psS.tile([128, 1024], F32, tag="sc",
                                     name=f"sc{g}")
                    for j in range(2):
                        nc.tensor.matmul(
                            sc_ps[:, j * 512:(j + 1) * 512],
                            ktp[t][hp * 64:hp * 64 + 64,
                                   cp * 128:(cp + 1) * 128],
                            qt_sb[t][hp * 64:hp * 64 + 64,
                                     qh * 1024 + j * 512:
                                     qh * 1024 + (j + 1) * 512],
                            start=True, stop=True,
                        )
                    return sc_ps

                def section_end(h, qh, ctx_ps):
                    t, hp = h // 2, h % 2
                    stg = pb.tile([DK + 1, 1024], F32, tag="stg",
                                  name=f"stg{h}_{qh}", bufs=3)
                    for j in range(2):
                        nc.vector.tensor_copy(
                            stg[:, j * 512:(j + 1) * 512], ctx_ps[j])
                    # reciprocal of rowsum via DRAM scatter to 64 partitions
                    # ([1,1024] single-lane DVE reciprocal is ~6.5us; this
                    # chain is ~3us and hidden under the next section).
                    rc_dr = dramB.tile([1, 1024], F32, tag="rc_dr",
                                       name=f"rcdr{h}_{qh}")
                    # single SBUF->SBUF partition-scatter DMA (dst/src APs
                    # iterate element-wise) replaces the two-hop DRAM bounce
                    rs64 = pb.tile([64, 16], F32, tag="rs64",
                                   name=f"rs64{h}_{qh}")
                    nc.sync.dma_start(rs64, stg[DK:DK + 1, :])
                    rc64 = pb.tile([64, 16], F32, tag="rc64",
                                   name=f"rc64{h}_{qh}")
                    nc.vector.reciprocal(rc64, rs64)
                    nc.sync.dma_start(
                        rc_dr.rearrange("o (p f) -> (o p) f", f=16), rc64)
                    rb = pb.tile([64, 1024], F32, tag="rb",
                                 name=f"rb{h}_{qh}")
                    # halves pipelined: first mul starts after half the
                    # broadcast transfer (matters for the final section,
                    # whose chain gates the tail)
                    for j in range(2):
                        nc.sync.dma_start(
                            rb[:, j * 512:(j + 1) * 512],
                            rc_dr[:, j * 512:(j + 1) * 512]
                            .to_broadcast([64, 512]))
                        nc.vector.tensor_mul(
                            ctx_sb[t][hp * 64:hp * 64 + 64,
                                      qh * 1024 + j * 512:
                                      qh * 1024 + (j + 1) * 512],
                            stg[0:DK, j * 512:(j + 1) * 512],
                            rb[:, j * 512:(j + 1) * 512],
                        )

                ctx_ps = None
                sc_cur = emit_scores(0)
                for g in range(NG):
                    sec, cp = divmod(g, CP)
                    h, qh = SECTIONS[sec]
                    if cp == 0:
                        ctx_ps = [psC.tile([DK + 1, 512], F32, tag="ctx",
                                           name=f"cx{sec}_{j}")
                                  for j in range(2)]
                    sc_next = emit_scores(g + 1) if g + 1 < NG else None
                    ex = pb.tile([128, 1024], MMDT, tag="ex",
                                 name=f"ex{g}", bufs=4)
                    nc.scalar.activation(ex, sc_cur, AF.Exp, scale=0.125)
                    for job in schedule[g]:
                        job()
                    for j in range(2):
                        nc.tensor.matmul(
                            ctx_ps[j],
                            vstk[:, h, cp, :],
                            ex[:, j * 512:(j + 1) * 512],
                            start=(cp == 0), stop=(cp == CP - 1),
                        )
                    if cp == CP - 1:
                        section_end(h, qh, ctx_ps)
                    sc_cur = sc_next

              # ---- tail: output projection for q-half 1 ----
              # Separate psum pool (attention pools closed -> banks free).
              # All eight tiles' t=0 matmuls run during the final normalize
              # chain (PE busy/warm); tiles 8-11 stay resident in psum,
              # tiles 12-15 stage to SBUF via the now-idle ACT engine and
              # re-accumulate with a DVE add after t=1.
              with tc.tile_pool(name="psT", bufs=4, space="PSUM") as psT:
                def t_mm(yp, t, qt_i, start, stop):
                    for n in range(2):
                        nc.tensor.matmul(
                            yp[:, n * 512:(n + 1) * 512],
                            ctx_sb[t][:, qt_i * 128:(qt_i + 1) * 128],
                            wo_sb[t][:, n * 512:(n + 1) * 512],
                            start=start, stop=stop,
                        )

                yps = {}
                y0_sb = {}
                for qt_i in range(8, 10):
                    yp = psT.tile([128, 1024], F32, tag="yt",
                                  name=f"yt{qt_i}", bufs=2)
                    yps[qt_i] = yp
                    t_mm(yp, 0, qt_i, start=True, stop=False)
                for qt_i in range(10, 16):
                    yp = psT.tile([128, 1024], F32, tag="yt0",
                                  name=f"yt0_{qt_i}", bufs=2)
                    t_mm(yp, 0, qt_i, start=True, stop=True)
                    ys0 = pb.tile([128, 1024], F32, tag="ys0",
                                  name=f"ys0_{qt_i}", bufs=6)
                    y0_sb[qt_i] = ys0
                    if qt_i % 2 == 0:
                        nc.scalar.copy(ys0, yp)
                    else:
                        nc.vector.tensor_copy(ys0, yp)

                # tiles 5-7 (q-half0, all ctx ready) fill the remaining
                # final-chain window with complete tiles
                for i, qt_i in enumerate(range(5, 8)):
                    yp = psT.tile([128, 1024], F32, tag="yt0",
                                  name=f"yh0_{qt_i}", bufs=2)
                    t_mm(yp, 0, qt_i, start=True, stop=False)
                    t_mm(yp, 1, qt_i, start=False, stop=True)
                    ys = pb.tile([128, 1024], F32, tag="ys",
                                 name=f"ys{qt_i}", bufs=6)
                    if i % 2 == 0:
                        nc.scalar.copy(ys, yp)
                    else:
                        nc.vector.tensor_copy(ys, yp)
                    rows = slice(qt_i * 128, (qt_i + 1) * 128)
                    nc.sync.dma_start(y[rows, 0:512], ys[:, 0:512])
                    nc.gpsimd.dma_start(y[rows, 512:1024], ys[:, 512:1024])

                for i, qt_i in enumerate(range(8, 10)):
                    yp = yps[qt_i]
                    t_mm(yp, 1, qt_i, start=False, stop=True)
                    ys = pb.tile([128, 1024], F32, tag="ys",
                                 name=f"ys{qt_i}", bufs=6)
                    if i % 2 == 0:
                        nc.scalar.copy(ys, yp)
                    else:
                        nc.vector.tensor_copy(ys, yp)
                    rows = slice(qt_i * 128, (qt_i + 1) * 128)
                    nc.sync.dma_start(y[rows, 0:512], ys[:, 0:512])
                    nc.gpsimd.dma_start(y[rows, 512:1024], ys[:, 512:1024])
                for i, qt_i in enumerate(range(10, 16)):
                    yp = psT.tile([128, 1024], F32, tag="yt0",
                                  name=f"yt1_{qt_i}", bufs=2)
                    t_mm(yp, 1, qt_i, start=True, stop=True)
                    ys = pb.tile([128, 1024], F32, tag="ys",
                                 name=f"ys{qt_i}", bufs=6)
                    nc.vector.tensor_add(ys, yp, y0_sb[qt_i])
                    rows = slice(qt_i * 128, (qt_i + 1) * 128)
                    nc.sync.dma_start(y[rows, 0:512], ys[:, 0:512])
                    nc.gpsimd.dma_start(y[rows, 512:1024], ys[:, 512:1024])
    _dedup_ldweights(nc)
    _legalize_matmul_waits(nc)
    return nc


_NC_CACHE = None


def _get_nc():
    global _NC_CACHE
    if _NC_CACHE is None:
        _NC_CACHE = build_nc()
    return _NC_CACHE


def make_in_maps(inputs):
    mmnp = mybir.dt.np(MMDT)
    x = np.asarray(inputs["x"], dtype=np.float32)
    Wq = np.asarray(inputs["Wq"], dtype=np.float32)
    Wk = np.asarray(inputs["Wk"], dtype=np.float32)
    Wv = np.asarray(inputs["Wv"], dtype=np.float32)
    Wo = np.asarray(inputs["Wo"], dtype=np.float32)
    bq = np.asarray(inputs["bq"], dtype=np.float32)
    bk = np.asarray(inputs["bk"], dtype=np.float32)

    in_maps = []
    for c in range(N_CORES):
        b, g = c // 4, c % 4
        sl = slice(g * E, (g + 1) * E)
        in_maps.append({
            "xt": np.ascontiguousarray(x[b].T).astype(mmnp),
            "wq": np.ascontiguousarray(Wq[:, sl]).astype(mmnp),
            "wk": np.ascontiguousarray(Wk[:, sl]).astype(mmnp),
            "wv": np.ascontiguousarray(Wv[:, sl]).astype(mmnp),
            "wo": np.ascontiguousarray(Wo[sl, :]).astype(mmnp),
            "bq": np.ascontiguousarray(bq[sl]),
            "bk": np.ascontiguousarray(bk[sl]),
        })
    return in_maps


def kernel(x, Wq, bq, Wk, bk, Wv, bv, Wo, bo):
    from concourse.bass_utils import run_bass_kernel_spmd

    x = np.asarray(x, dtype=np.float32)
    Wv = np.asarray(Wv, dtype=np.float32)
    Wo = np.asarray(Wo, dtype=np.float32)
    bv = np.asarray(bv, dtype=np.float32)
    bo = np.asarray(bo, dtype=np.float32)

    B = x.shape[0]
    nc = _get_nc()
    in_maps = make_in_maps({
        "x": x, "Wq": Wq, "Wk": Wk, "Wv": Wv, "Wo": Wo, "bq": bq, "bk": bk,
    })

    res = run_bass_kernel_spmd(nc, in_maps, core_ids=list(range(N_CORES)))

    bias_total = bo + bv @ Wo  # [D]
    out = np.zeros((B, S, D), dtype=np.float32)
    for c in range(N_CORES):
        out[c // 4] += res.results[c]["y"]
    out += bias_total[None, None, :]
    return out


# revision 38
# speedup vs baseline: 1.0007x; 1.0001x over previous
"""Multi-head self-attention (B=2, S=2048, D=1024, H=16) on 8 trn2 cores.

Sharding: core c = b*4 + g  (b = batch, g = head-group of 4 heads).
Each core computes, for its batch b and heads 4g..4g+3:
  Qt/Kt = W^T x_b^T + bias   -> [128, 2048] feature-major per head pair
  V||ones                    -> vstk [128 keys, h, cp, 65]
  scoresT[k,q] per head      -> [128 keys, 1024 q] psum ([64,128] Kt weights)
  expT = exp(0.125*scoresT)  (ACT, straight from PSUM)
  ctxT/rowsum via PV matmul with [128, 65] V||ones weights (contract 128 keys)
  ctxT normalized by 1/rowsum (DMA partition-broadcast + DVE mult)
  y = ctx @ Wo               -> [2048, 1024]
One globally software-pipelined attention loop (128 iterations); pair-1
Q/K projections and the first-half output projection interleave into the
ACT-bound exp stream; tail output projection overlaps the last normalize
chain. Host: Y[b] = sum_g y_partial + (bo + bv @ Wo).
"""

import sys

sys.path.insert(0, "/opt/trn_rl_repo")

import numpy as np

import concourse.bass as bass
import concourse.mybir as mybir
import concourse.tile as tile

F32 = mybir.dt.float32
F16 = mybir.dt.float16
MMDT = F16                     # dtype for all matmul operands
AF = mybir.ActivationFunctionType

D = 1024          # d_model
S = 2048          # sequence length
HPC = 4           # heads per core
DK = 64           # head dim
E = HPC * DK      # 256 features per core
N_CORES = 8

KT = D // 128     # 8 k-tiles over d_model
CP = S // 128     # 16 key chunks of 128
ET = 2            # head pairs

# attention sections: (head, q-half), qh-outer
SECTIONS = [(h, qh) for qh in range(2) for h in range(HPC)]
NG = len(SECTIONS) * CP        # 128 global iterations


_ENGINE_OPS = {
    "InstMatmult", "InstActivation", "InstTensorCopy", "InstTensorTensor",
    "InstReciprocal", "InstTensorReduce", "InstMemset", "InstIota",
    "InstTensorScalarPtr", "InstTranspose", "InstLdweights",
    "InstDMACopy", "InstDrain", "InstNoOp",
}


def _dedup_ldweights(nc):
    """Consecutive matmuls often reuse identical weights (score j-halves, PV
    j-halves, proj q-chunks). The rust layer emits one standalone
    InstLdweights per matmul; reloading identical weights costs ~100ns of PE
    array drain each. Drop an InstLdweights when the previously loaded
    weights AP is byte-identical (weight tiles here are write-once), folding
    its waits into the next matmul (legalized afterwards)."""

    def key(ap):
        return (ap.memref, ap.offset, str(ap.ap), str(ap.dtype))

    n_drop = 0
    for f in nc.m.functions:
        for bb in f.blocks:
            out = []
            pending_waits = []
            last_w = None
            for i in bb.instructions:
                if type(i).__name__ == "InstLdweights":
                    k = key(i.ins[0])
                    if k == last_w:
                        si = getattr(i, "sync_info", None)
                        if si is not None and si.on_wait:
                            pending_waits.extend(si.on_wait)
                        n_drop += 1
                        continue
                    last_w = k
                elif pending_waits and type(i).__name__ == "InstMatmult":
                    si = getattr(i, "sync_info", None)
                    waits = list(si.on_wait) if si else []
                    upd = list(si.on_update) if si else []
                    i.sync_info = mybir.SyncInfo(
                        on_wait=pending_waits + waits, on_update=upd)
                    pending_waits = []
                out.append(i)
            assert not pending_waits
            bb.instructions = out
    return n_drop


def _legalize_matmul_waits(nc):
    """walrus allows at most 1 sync wait on engine compute instructions; Tile
    sometimes emits more. Move the excess onto EventSemaphore instructions
    (cap 2 each) placed immediately before in same-engine program order."""
    for f in nc.m.functions:
        for bb in f.blocks:
            out = []
            changed = False
            for i in bb.instructions:
                si = getattr(i, "sync_info", None)
                if (
                    type(i).__name__ in _ENGINE_OPS
                    and si is not None
                    and si.on_wait
                    and len(si.on_wait) > 1
                ):
                    waits = list(si.on_wait)
                    excess, keep = waits[:-1], waits[-1:]
                    for c in range(0, len(excess), 2):
                        ev = mybir.InstEventSemaphore(
                            name=f"{i.name}-mmw{c}", ins=[], outs=[]
                        )
                        ev.engine = i.engine
                        ev.sync_info = mybir.SyncInfo(
                            on_wait=excess[c:c + 2], on_update=[]
                        )
                        out.append(ev)
                    i.sync_info = mybir.SyncInfo(
                        on_wait=keep, on_update=list(si.on_update)
                    )
                    changed = True
                out.append(i)
            if changed:
                bb.instructions = out


def build_nc():
    nc = bass.Bass()

    xt = nc.dram_tensor("xt", [D, S], MMDT, kind="ExternalInput")
    wq = nc.dram_tensor("wq", [D, E], MMDT, kind="ExternalInput")
    wk = nc.dram_tensor("wk", [D, E], MMDT, kind="ExternalInput")
    wv = nc.dram_tensor("wv", [D, E], MMDT, kind="ExternalInput")
    wo = nc.dram_tensor("wo", [E, D], MMDT, kind="ExternalInput")
    bq = nc.dram_tensor("bq", [E], F32, kind="ExternalInput")
    bk = nc.dram_tensor("bk", [E], F32, kind="ExternalInput")
    y = nc.dram_tensor("y", [S, D], F32, kind="ExternalOutput")

    with tile.TileContext(nc) as tc:
        with tc.tile_pool(name="persist", bufs=1) as pp:
            # ---- persistent tiles ----
            qt_sb = [pp.tile([128, S], MMDT, tag=f"qt{t}", name=f"qt{t}")
                     for t in range(ET)]
            ktp = [pp.tile([128, S], MMDT, tag=f"ktp{t}", name=f"ktp{t}")
                   for t in range(ET)]
            # vstk[:, h, cp, 0:64] = V rows for head h, keys cp*128..+128
            # (partition = key within chunk); [:, h, cp, 64] = 1.0 (rowsum).
            vstk = pp.tile([128, HPC, CP, DK + 1], MMDT, tag="vstk")
            ctx_sb = [pp.tile([128, S], MMDT, tag=f"ctx{t}", name=f"ctx{t}")
                      for t in range(ET)]
            wo_sb = [pp.tile([128, D], MMDT, tag=f"wo{t}", name=f"wo{t}")
                     for t in range(ET)]
            bq_sb = pp.tile([128, ET], F32, tag="bq")
            bk_sb = pp.tile([128, ET], F32, tag="bk")
            xt_sb = [pp.tile([128, S], MMDT, tag=f"xt{k}", name=f"xt{k}")
                     for k in range(KT)]
            wq_sb = pp.tile([128, KT, E], MMDT, tag="wq")
            wk_sb = pp.tile([128, KT, E], MMDT, tag="wk")
            wv_sb = pp.tile([128, KT, E], MMDT, tag="wv")

            nc.gpsimd.dma_start(bq_sb, bq.rearrange("(t p) -> p t", p=128))
            nc.gpsimd.dma_start(bk_sb, bk.rearrange("(t p) -> p t", p=128))
            # k-layer inputs arrive together so QK0 proj chases the DMAs;
            # issues spread across four queues for parallel transfer.
            for k in range(KT):
                nc.scalar.dma_start(wq_sb[:, k, :],
                                    wq[k * 128:(k + 1) * 128, :])
                nc.scalar.dma_start(wk_sb[:, k, :],
                                    wk[k * 128:(k + 1) * 128, :])
                nc.sync.dma_start(xt_sb[k][:, 0:1024],
                                  xt[k * 128:(k + 1) * 128, 0:1024])
                nc.gpsimd.dma_start(xt_sb[k][:, 1024:2048],
                                    xt[k * 128:(k + 1) * 128, 1024:2048])
            for k in range(KT):
                nc.scalar.dma_start(wv_sb[:, k, :],
                                    wv[k * 128:(k + 1) * 128, :])
            for t in range(ET):
                nc.scalar.dma_start(wo_sb[t], wo[t * 128:(t + 1) * 128, :])

            ones_sb = pp.tile([128, CP], F32, tag="ones")
            nc.vector.memset(ones_sb, 1.0)
            for h in range(HPC):
                nc.vector.tensor_copy(vstk[:, h, :, DK:DK + 1],
                                      ones_sb[:, :, None])
            # warm the ACT function table during the DMA dead time; the lazy
            # load (1.3us) otherwise lands on phase A's eviction critical
            # path. Must use the same scale/bias-enabled Identity variant as
            # the evictions (plain Identity loads a different table set).
            tbl_warm = pp.tile([128, 16], F32, tag="tblw")
            nc.scalar.activation(tbl_warm, ones_sb[:, 0:16], AF.Identity,
                                 bias=ones_sb[:, 0:1])

            # ---- stage A: pair-0 Q/K projections + V (all heads) ----
            with tc.tile_pool(name="psA", bufs=4, space="PSUM") as psA:
                # 4 resident [128,1024] psums (8 banks), k-outer so matmuls
                # chase the input DMAs.
                qps = [psA.tile([128, 1024], F32, tag="proj", name=f"qp{i}")
                       for i in range(2)]
                kps = [psA.tile([128, 1024], F32, tag="proj", name=f"kp{i}")
                       for i in range(2)]
                for k in range(KT):
                    for half in range(2):
                        for j in range(2):
                            q0 = half * 1024 + j * 512
                            nc.tensor.matmul(
                                qps[half][:, j * 512:(j + 1) * 512],
                                wq_sb[:, k, 0:128],
                                xt_sb[k][:, q0:q0 + 512],
                                start=(k == 0), stop=(k == KT - 1),
                            )
                    for half in range(2):
                        for j in range(2):
                            q0 = half * 1024 + j * 512
                            nc.tensor.matmul(
                                kps[half][:, j * 512:(j + 1) * 512],
                                wk_sb[:, k, 0:128],
                                xt_sb[k][:, q0:q0 + 512],
                                start=(k == 0), stop=(k == KT - 1),
                            )
                # evictions with bias on ACT (idle during stage A)
                for half in range(2):
                    nc.scalar.activation(
                        qt_sb[0][:, half * 1024:(half + 1) * 1024],
                        qps[half], AF.Identity, bias=bq_sb[:, 0:1])
                for half in range(2):
                    nc.scalar.activation(
                        ktp[0][:, half * 1024:(half + 1) * 1024],
                        kps[half], AF.Identity, bias=bk_sb[:, 0:1])

                # V for all 4 heads, seq-major: psum [128 seq, 256 feat]
                for s in range(CP):
                    vp = psA.tile([128, 1024], F32, tag="proj",
                                  name=f"vp{s}")
                    for k in range(KT):
                        nc.tensor.matmul(
                            vp[:, 0:E],
                            xt_sb[k][:, s * 128:(s + 1) * 128],
                            wv_sb[:, k, :],
                            start=(k == 0), stop=(k == KT - 1),
                        )
                    nc.vector.tensor_copy(
                        vstk[:, :, s, 0:DK],
                        vp[:, 0:E].rearrange("p (h d) -> p h d", d=DK))

            # ---- stage B: one pipelined attention loop ----
            with (
                tc.tile_pool(name="stageB", bufs=3) as pb,
                tc.tile_pool(name="dramB", bufs=3, space="DRAM") as dramB,
            ):
              with (
                tc.tile_pool(name="psS", bufs=2, space="PSUM") as psS,
                tc.tile_pool(name="psC", bufs=2, space="PSUM") as psC,
                tc.tile_pool(name="psX", bufs=2, space="PSUM") as psX,
              ):
                # --- feeder jobs: closures emitting one PE matmul each ---
                def proj1_jobs(w_sb, b_col, dst, chunks):
                    """Pair-1 proj matmuls, k-inner per [128,512] chunk."""
                    jobs = []
                    for ch in chunks:
                        ps = {}

                        def mk(k, ch=ch, ps=ps):
                            def go():
                                if k == 0:
                                    ps[0] = psX.tile(
                                        [128, 512], F32, tag="aux",
                                        name=f"pj{id(w_sb)}_{ch}")
                                nc.tensor.matmul(
                                    ps[0],
                                    w_sb[:, k, 128:256],
                                    xt_sb[k][:, ch * 512:(ch + 1) * 512],
                                    start=(k == 0), stop=(k == KT - 1),
                                )
                                if k == KT - 1:
                                    nc.vector.tensor_scalar_add(
                                        dst[:, ch * 512:(ch + 1) * 512],
                                        ps[0], b_col)
                            return go
                        for k in range(KT):
                            jobs.append(mk(k))
                    return jobs

                def outproj_jobs(tiles):
                    """Output projection per 128-q tile: 4 matmuls + evict
                    + y DMA."""
                    jobs = []
                    for qt_i in tiles:
                        st = {}

                        def mk(t, n, qt_i=qt_i, st=st):
                            def go():
                                if (t, n) == (0, 0):
                                    st['ps'] = [
                                        psX.tile([128, 512], F32, tag="aux",
                                                 name=f"yp{qt_i}_{j}")
                                        for j in range(2)]
                                    st['ys'] = pb.tile(
                                        [128, 1024], F32, tag="ys",
                                        name=f"ys{qt_i}", bufs=6)
                                nc.tensor.matmul(
                                    st['ps'][n],
                                    ctx_sb[t][:, qt_i * 128:(qt_i + 1) * 128],
                                    wo_sb[t][:, n * 512:(n + 1) * 512],
                                    start=(t == 0), stop=(t == ET - 1),
                                )
                                if t == ET - 1:
                                    nc.vector.tensor_copy(
                                        st['ys'][:, n * 512:(n + 1) * 512],
                                        st['ps'][n])
                                if (t, n) == (1, 1):
                                    nc.sync.dma_start(
                                        y[qt_i * 128:(qt_i + 1) * 128, :],
                                        st['ys'])
                            return go
                        for t in range(ET):
                            for n in range(2):
                                jobs.append(mk(t, n))
                    return jobs

                # schedule feeder jobs onto global iterations
                schedule = [[] for _ in range(NG)]

                def assign(g_lo, g_hi, jobs):
                    n_slots = g_hi - g_lo
                    acc = 0.0
                    i = 0
                    per = len(jobs) / n_slots
                    for g in range(g_lo, g_hi):
                        acc += per
                        take = int(round(acc)) - i
                        schedule[g] = jobs[i:i + take]
                        i += take
                    assert i == len(jobs)

                # Q1 half0 + K1 during sections (0,0),(1,0); order matters:
                # section (2,0)'s first scores (emitted at g=31) need the
                # Q1-h0 and K1-ch0 evictions done well before.
                assign(0, 32,
                       proj1_jobs(wq_sb, bq_sb[:, 1:2], qt_sb[1], range(2))
                       + proj1_jobs(wk_sb, bk_sb[:, 1:2], ktp[1], range(4)))
                # Q1 half1 during (2,0),(3,0)
                assign(32, 64,
                       proj1_jobs(wq_sb, bq_sb[:, 1:2], qt_sb[1],
                                  range(2, 4)))
                # outproj q-half0 tiles 0-4 during (0,1),(1,1); offset past
                # (3,0)'s normalize chain (its ctx lands a few us into
                # (0,1)). Tiles 5-7 fill the final normalize-chain gap in
                # the tail instead.
                assign(72, 96, outproj_jobs(range(5)))

                def emit_scores(g):
                    sec, cp = divmod(g, CP)
                    h, qh = SECTIONS[sec]
                    t, hp = h // 2, h % 2
                    sc_ps = psS.tile([128, 1024], F32, tag="sc",
                                     name=f"sc{g}")
                    for j in range(2):
                        nc.tensor.matmul(
                            sc_ps[:, j * 512:(j + 1) * 512],
                            ktp[t][hp * 64:hp * 64 + 64,
                                   cp * 128:(cp + 1) * 128],
                            qt_sb[t][hp * 64:hp * 64 + 64,
                                     qh * 1024 + j * 512:
                                     qh * 1024 + (j + 1) * 512],
                            start=True, stop=True,
                        )
                    return sc_ps

                def section_end(h, qh, ctx_ps):
                    t, hp = h // 2, h % 2
                    stg = pb.tile([DK + 1, 1024], F32, tag="stg",
                                  name=f"stg{h}_{qh}", bufs=3)
                    for j in range(2):
                        nc.vector.tensor_copy(
                            stg[:, j * 512:(j + 1) * 512], ctx_ps[j])
                    # reciprocal of rowsum via DRAM scatter to 64 partitions
                    # ([1,1024] single-lane DVE reciprocal is ~6.5us; this
                    # chain is ~3us and hidden under the next section).
                    rc_dr = dramB.tile([1, 1024], F32, tag="rc_dr",
                                       name=f"rcdr{h}_{qh}")
                    # single SBUF->SBUF partition-scatter DMA (dst/src APs
                    # iterate element-wise) replaces the two-hop DRAM bounce
                    rs64 = pb.tile([64, 16], F32, tag="rs64",
                                   name=f"rs64{h}_{qh}")
                    nc.sync.dma_start(rs64, stg[DK:DK + 1, :])
                    rc64 = pb.tile([64, 16], F32, tag="rc64",
                                   name=f"rc64{h}_{qh}")
                    nc.vector.reciprocal(rc64, rs64)
                    nc.sync.dma_start(
                        rc_dr.rearrange("o (p f) -> (o p) f", f=16), rc64)
                    rb = pb.tile([64, 1024], F32, tag="rb",
                                 name=f"rb{h}_{qh}")
                    # halves pipelined: first mul starts after half the
                    # broadcast transfer (matters for the final section,
                    # whose chain gates the tail)
                    for j in range(2):
                        nc.sync.dma_start(
                            rb[:, j * 512:(j + 1) * 512],
                            rc_dr[:, j * 512:(j + 1) * 512]
                            .to_broadcast([64, 512]))
                        nc.vector.tensor_mul(
                            ctx_sb[t][hp * 64:hp * 64 + 64,
                                      qh * 1024 + j * 512:
                                      qh * 1024 + (j + 1) * 512],
                            stg[0:DK, j * 512:(j + 1) * 512],
                            rb[:, j * 512:(j + 1) * 512],
                        )

                ctx_ps = None
                sc_cur = emit_scores(0)
                for g in range(NG):
                    sec, cp = divmod(g, CP)
                    h, qh = SECTIONS[sec]
                    if cp == 0:
                        ctx_ps = [psC.tile([DK + 1, 512], F32, tag="ctx",
                                           name=f"cx{sec}_{j}")
                                  for j in range(2)]
                    sc_next = emit_scores(g + 1) if g + 1 < NG else None
                    ex = pb.tile([128, 1024], MMDT, tag="ex",
                                 name=f"ex{g}", bufs=4)
                    nc.scalar.activation(ex, sc_cur, AF.Exp, scale=0.125)
                    for job in schedule[g]:
                        job()
                    for j in range(2):
                        nc.tensor.matmul(
                            ctx_ps[j],
                            vstk[:, h, cp, :],
                            ex[:, j * 512:(j + 1) * 512],
                            start=(cp == 0), stop=(cp == CP - 1),
                        )
                    if cp == CP - 1:
                        section_end(h, qh, ctx_ps)
                    sc_cur = sc_next

              # ---- tail: output projection for q-half 1 ----
              # Separate psum pool (attention pools closed -> banks free).
              # All eight tiles' t=0 matmuls run during the final normalize
              # chain (PE busy/warm); tiles 8-11 stay resident in psum,
              # tiles 12-15 stage to SBUF via the now-idle ACT engine and
              # re-accumulate with a DVE add after t=1.
              with tc.tile_pool(name="psT", bufs=4, space="PSUM") as psT:
                def t_mm(yp, t, qt_i, start, stop):
                    for n in range(2):
                        nc.tensor.matmul(
                            yp[:, n * 512:(n + 1) * 512],
                            ctx_sb[t][:, qt_i * 128:(qt_i + 1) * 128],
                            wo_sb[t][:, n * 512:(n + 1) * 512],
                            start=start, stop=stop,
                        )

                yps = {}
                y0_sb = {}
                for qt_i in range(8, 10):
                    yp = psT.tile([128, 1024], F32, tag="yt",
                                  name=f"yt{qt_i}", bufs=2)
                    yps[qt_i] = yp
                    t_mm(yp, 0, qt_i, start=True, stop=False)
                for qt_i in range(10, 16):
                    yp = psT.tile([128, 1024], F32, tag="yt0",
                                  name=f"yt0_{qt_i}", bufs=2)
                    t_mm(yp, 0, qt_i, start=True, stop=True)
                    ys0 = pb.tile([128, 1024], F32, tag="ys0",
                                  name=f"ys0_{qt_i}", bufs=6)
                    y0_sb[qt_i] = ys0
                    if qt_i % 2 == 0:
                        nc.scalar.copy(ys0, yp)
                    else:
                        nc.vector.tensor_copy(ys0, yp)

                # tiles 5-7 (q-half0, all ctx ready) fill the remaining
                # final-chain window with complete tiles
                for i, qt_i in enumerate(range(5, 8)):
                    yp = psT.tile([128, 1024], F32, tag="yt0",
                                  name=f"yh0_{qt_i}", bufs=2)
                    t_mm(yp, 0, qt_i, start=True, stop=False)
                    t_mm(yp, 1, qt_i, start=False, stop=True)
                    ys = pb.tile([128, 1024], F32, tag="ys",
                                 name=f"ys{qt_i}", bufs=6)
                    if i % 2 == 0:
                        nc.scalar.copy(ys, yp)
                    else:
                        nc.vector.tensor_copy(ys, yp)
                    rows = slice(qt_i * 128, (qt_i + 1) * 128)
                    nc.sync.dma_start(y[rows, 0:512], ys[:, 0:512])
                    nc.gpsimd.dma_start(y[rows, 512:1024], ys[:, 512:1024])

                for i, qt_i in enumerate(range(8, 10)):
                    yp = yps[qt_i]
                    t_mm(yp, 1, qt_i, start=False, stop=True)
                    ys = pb.tile([128, 1024], F32, tag="ys",
                                 name=f"ys{qt_i}", bufs=6)
                    if i % 2 == 0:
                        nc.scalar.copy(ys, yp)
                    else:
                        nc.vector.tensor_copy(ys, yp)
                    rows = slice(qt_i * 128, (qt_i + 1) * 128)
                    qs = (nc.sync, nc.gpsimd, nc.scalar)
                    qs[qt_i % 3].dma_start(y[rows, 0:512], ys[:, 0:512])
                    qs[(qt_i + 1) % 3].dma_start(y[rows, 512:1024],
                                                 ys[:, 512:1024])
                for i, qt_i in enumerate(range(10, 16)):
                    yp = psT.tile([128, 1024], F32, tag="yt0",
                                  name=f"yt1_{qt_i}", bufs=2)
                    t_mm(yp, 1, qt_i, start=True, stop=True)
                    ys = pb.tile([128, 1024], F32, tag="ys",
                                 name=f"ys{qt_i}", bufs=6)
                    nc.vector.tensor_add(ys, yp, y0_sb[qt_i])
                    rows = slice(qt_i * 128, (qt_i + 1) * 128)
                    qs = (nc.sync, nc.gpsimd, nc.scalar)
                    qs[qt_i % 3].dma_start(y[rows, 0:512], ys[:, 0:512])
                    qs[(qt_i + 1) % 3].dma_start(y[rows, 512:1024],
                                                 ys[:, 512:1024])
    _dedup_ldweights(nc)
    _legalize_matmul_waits(nc)
    return nc


_NC_CACHE = None


def _get_nc():
    global _NC_CACHE
    if _NC_CACHE is None:
        _NC_CACHE = build_nc()
    return _NC_CACHE


def make_in_maps(inputs):
    mmnp = mybir.dt.np(MMDT)
    x = np.asarray(inputs["x"], dtype=np.float32)
    Wq = np.asarray(inputs["Wq"], dtype=np.float32)
    Wk = np.asarray(inputs["Wk"], dtype=np.float32)
    Wv = np.asarray(inputs["Wv"], dtype=np.float32)
    Wo = np.asarray(inputs["Wo"], dtype=np.float32)
    bq = np.asarray(inputs["bq"], dtype=np.float32)
    bk = np.asarray(inputs["bk"], dtype=np.float32)

    in_maps = []
    for c in range(N_CORES):
        b, g = c // 4, c % 4
        sl = slice(g * E, (g + 1) * E)
        in_maps.append({
            "xt": np.ascontiguousarray(x[b].T).astype(mmnp),
            "wq": np.ascontiguousarray(Wq[:, sl]).astype(mmnp),
            "wk": np.ascontiguousarray(Wk[:, sl]).astype(mmnp),
            "wv": np.ascontiguousarray(Wv[:, sl]).astype(mmnp),
            "wo": np.ascontiguousarray(Wo[sl, :]).astype(mmnp),
            "bq": np.ascontiguousarray(bq[sl]),
            "bk": np.ascontiguousarray(bk[sl]),
        })
    return in_maps


def kernel(x, Wq, bq, Wk, bk, Wv, bv, Wo, bo):
    from concourse.bass_utils import run_bass_kernel_spmd

    x = np.asarray(x, dtype=np.float32)
    Wv = np.asarray(Wv, dtype=np.float32)
    Wo = np.asarray(Wo, dtype=np.float32)
    bv = np.asarray(bv, dtype=np.float32)
    bo = np.asarray(bo, dtype=np.float32)

    B = x.shape[0]
    nc = _get_nc()
    in_maps = make_in_maps({
        "x": x, "Wq": Wq, "Wk": Wk, "Wv": Wv, "Wo": Wo, "bq": bq, "bk": bk,
    })

    res = run_bass_kernel_spmd(nc, in_maps, core_ids=list(range(N_CORES)))

    bias_total = bo + bv @ Wo  # [D]
    out = np.zeros((B, S, D), dtype=np.float32)
    for c in range(N_CORES):
        out[c // 4] += res.results[c]["y"]
    out += bias_total[None, None, :]
    return out
